# revision 32
# baseline (speedup 1.0000x reference)
"""Trainium2 Bass kernel for nn_DecoderLayer (gnn_message_passing).

Per-node grouped 3x3 conv over 16 gathered parent maps, + bias, tanh,
activity gating.  K=1024 nodes sharded 128/core across 8 cores.

Per-core strategy (128 nodes, 16 groups of 8):
  - indirect-DMA gather of the group's 128 (node,slot) parent maps
    (each 64x64 f32, contiguous 16KB) into SBUF G (128, 4098) at col 1.
  - dx-folded block-diagonal matmul: 3 accumulating matmuls (dx=-1,0,1)
    with host-packed lhsT_dx (128, 24) [col = (node, dy)], rhs = G
    shifted by dx in the flat free dim -> psum (24, 4096) holds
    row-convolved planes h[(n,dy)][x].
  - column wrap at j=0/63 fixed by 4 small correction matmuls (N=64)
    whose results overwrite those columns after the psum->SBUF copy.
  - copy psum -> H (24, 4096) on ACT/DVE, then 3 SBUF->SBUF DMAs
    scatter H's dy-slices to per-dy planes (128 nodes, 4096).
  - epilogue: out = tanh((P0[x] + Pm1[x-64] + Pp1[x+64]) * mask + b*mask)
    -- the vertical taps are flat shifted adds on DVE; mask folded into
    the ACT scale so one activation does bias+tanh+gating.
"""

import sys

for _p in ("/opt/trn_rl_repo",):
    if _p not in sys.path:
        sys.path.insert(0, _p)

import numpy as np

import concourse.bass as bass
import concourse.mybir as mybir
from concourse.bass import IndirectOffsetOnAxis
from concourse.bass_utils import run_bass_kernel_spmd
import concourse.tile as _tile_mod
from concourse.tile import TileContext


_MAX_WAITS = 1  # this walrus build rejects multiple sync-waits per instruction


def _split_excess_waits(nc):
    """Workaround for "Too many sync wait commands": move excess on_wait
    entries onto same-engine nop instructions spliced just before the
    offending instruction (waits are order-insensitive preconditions)."""
    ctr = [0]
    for fn in nc.m.functions:
        for blk in fn.blocks:
            insts = list(blk.instructions)
            out = []
            changed = False
            for inst in insts:
                si = inst.sync_info
                waits = list(si.on_wait) if si is not None else []
                if len(waits) > _MAX_WAITS:
                    changed = True
                    excess = waits[:-_MAX_WAITS]
                    keep = waits[-_MAX_WAITS:]
                    while excess:
                        chunk, excess = excess[:_MAX_WAITS], excess[_MAX_WAITS:]
                        ctr[0] += 1
                        nop = mybir.InstNoOp(
                            name=f"I-waitsplit-{ctr[0]}",
                            engine=inst.engine,
                            sync_info=mybir.SyncInfo(on_wait=chunk, on_update=[]),
                        )
                        out.append(nop)
                    si.on_wait = keep
                    inst.sync_info = si
                out.append(inst)
            if changed:
                blk.instructions = out

# Problem constants (hardcoded per contract)
K_NODES = 1024
M_PREV = 1024
FAN_IN = 16
THRESHOLD = 12
NROW = 64
NCOL = 64
SP = NROW * NCOL  # 4096
N_CORES = 8
NPC = K_NODES // N_CORES  # nodes per core = 128
GROUPS = 16
GN = NPC // GROUPS  # nodes per group = 8
MCOL = GN * 3  # lhsT columns per group-dx = 24
CHUNK = 512
NCHUNK = SP // CHUNK  # 8

F32 = mybir.dt.float32
F32R = mybir.dt.float32r
I32 = mybir.dt.int32


def _build_program(split_waits=True, ablate=()):
    nc = bass.Bass(detect_race_conditions=False)
    prev = nc.dram_tensor("prev", [M_PREV, SP], F32R, kind="ExternalInput")
    gidx = nc.dram_tensor("gidx", [NPC, GROUPS], I32, kind="ExternalInput")
    lhst = nc.dram_tensor("lhst", [NPC, GROUPS * 3 * MCOL], F32R, kind="ExternalInput")
    biasm = nc.dram_tensor("biasm", [NPC, 1], F32, kind="ExternalInput")
    maskv = nc.dram_tensor("maskv", [NPC, 1], F32, kind="ExternalInput")
    out = nc.dram_tensor("out", [NPC, SP], F32, kind="ExternalOutput")

    with TileContext(nc) as tc:
        with (
            tc.tile_pool(name="const", bufs=1) as cpool,
            tc.tile_pool(name="g", bufs=3) as gpool,
            tc.tile_pool(name="h", bufs=3) as hpool,
            tc.tile_pool(name="planes", bufs=1) as ppool,
            tc.tile_pool(name="epi", bufs=1) as epool,
            tc.tile_pool(name="ps", bufs=5, space="PSUM") as pspool,
            tc.tile_pool(name="psc", bufs=2, space="PSUM") as pscpool,
        ):
            idx_sb = cpool.tile([NPC, GROUPS], I32, tag="idx")
            nc.sync.dma_start(out=idx_sb[:], in_=gidx[:])
            lhst_sb = cpool.tile([NPC, GROUPS * 3 * MCOL], F32R, tag="lhst")
            nc.sync.dma_start(out=lhst_sb[:], in_=lhst[:])
            biasm_sb = cpool.tile([NPC, 1], F32, tag="biasm")
            nc.sync.dma_start(out=biasm_sb[:], in_=biasm[:])
            mask_sb = cpool.tile([NPC, 1], F32, tag="maskv")
            nc.sync.dma_start(out=mask_sb[:], in_=maskv[:])

            # per-dy planes accumulated across groups
            plt = ppool.tile([NPC, 3 * SP], F32, tag="plt", name="plt")
            pl = [plt[:, d * SP : (d + 1) * SP] for d in range(3)]

            for g in range(GROUPS):
                G = gpool.tile([NPC, SP + 2], F32R, tag="G")
                nc.vector.memset(G[:, 0:1].bitcast(F32), 0.0)
                nc.vector.memset(G[:, SP + 1 : SP + 2].bitcast(F32), 0.0)
                if "gather" not in ablate:
                    nc.gpsimd.indirect_dma_start(
                        out=G[:, 1 : SP + 1],
                    out_offset=None,
                        in_=prev[:],
                        in_offset=IndirectOffsetOnAxis(ap=idx_sb[:, g : g + 1], axis=0),
                    )
                Gv = G[:, 1 : SP + 1].rearrange("p (r c) -> p r c", c=NCOL)

                H = hpool.tile([MCOL, SP], F32, tag="H")

                def lw(dxi):
                    c0 = (g * 3 + dxi) * MCOL
                    return lhst_sb[:, c0 : c0 + MCOL]

                # main dx-folded matmuls, chunked to 512-col psum tiles
                for c in range(NCHUNK):
                    ps = pspool.tile([MCOL, CHUNK], F32, tag="ps")
                    for dxi in range(3):
                        # rhs offset: data starts at col 1; dx = dxi-1
                        off = c * CHUNK + dxi
                        nc.tensor.matmul(
                            out=ps[:],
                            lhsT=lw(dxi),
                            rhs=G[:, off : off + CHUNK],
                            start=(dxi == 0),
                            stop=(dxi == 2),
                        )
                    if c % 8 < 2:
                        nc.scalar.copy(out=H[:, c * CHUNK : (c + 1) * CHUNK], in_=ps[:])
                    else:
                        nc.vector.tensor_copy(
                            out=H[:, c * CHUNK : (c + 1) * CHUNK], in_=ps[:]
                        )

                # correction matmuls for wrapped columns j=0 and j=63
                psc = pscpool.tile([MCOL, 2 * NROW], F32, tag="psc")
                nc.tensor.matmul(
                    out=psc[:, 0:NROW], lhsT=lw(1), rhs=Gv[:, :, 0], start=True, stop=False
                )
                nc.tensor.matmul(
                    out=psc[:, 0:NROW], lhsT=lw(2), rhs=Gv[:, :, 1], start=False, stop=True
                )
                nc.tensor.matmul(
                    out=psc[:, NROW : 2 * NROW],
                    lhsT=lw(1),
                    rhs=Gv[:, :, NCOL - 1],
                    start=True,
                    stop=False,
                )
                nc.tensor.matmul(
                    out=psc[:, NROW : 2 * NROW],
                    lhsT=lw(0),
                    rhs=Gv[:, :, NCOL - 2],
                    start=False,
                    stop=True,
                )
                Hv = H.rearrange("p (r c) -> p r c", c=NCOL)
                nc.vector.tensor_copy(out=Hv[:, :, 0], in_=psc[:, 0:NROW])
                nc.vector.tensor_copy(out=Hv[:, :, NCOL - 1], in_=psc[:, NROW : 2 * NROW])

                # scatter all 3 dy slices of H to the plane tensor in one DMA.
                # H rows are node-major (row = n*3 + d).  Plane partition for
                # (group g, node n) is p = 16n + g, so the 8 dst partitions
                # spread across 8 SBUF octets (8 SDMA engines) instead of 1.
                dma_eng = nc.sync if g % 2 == 0 else nc.scalar
                if "collect" in ablate:
                    dma_eng.dma_start(
                        out=plt[g * GN : (g + 1) * GN, 0:16], in_=H[0:GN, 0:16]
                    )
                else:
                    dma_eng.dma_start(
                        out=plt[g * GN : (g + 1) * GN, :].rearrange(
                            "n (d x) -> n d x", d=3
                        ),
                        in_=H[:, :],
                    )

            # epilogue: vertical taps + bias + tanh + mask.  Processed in two
            # node-halves so the first half (groups 0-7) overlaps with the
            # second half's compute instead of serializing after all collects.
            t1f = epool.tile([NPC, SP], F32, tag="t1", name="t1f")
            t2f = epool.tile([NPC, SP], F32, tag="t2", name="t2f")
            yf = epool.tile([NPC, SP], F32, tag="y", name="yf")
            for h0, h1 in ((0, NPC),):
                t1 = t1f[h0:h1, :]
                t2 = t2f[h0:h1, :]
                y = yf[h0:h1, :]
                P0 = pl[1][h0:h1, :]
                Pm1 = pl[0][h0:h1, :]
                Pp1 = pl[2][h0:h1, :]
                # t1 = P0 + shift_down(Pm1):  t1[x] = P0[x] + Pm1[x-64]
                nc.vector.tensor_add(
                    out=t1[:, NCOL:SP], in0=P0[:, NCOL:SP], in1=Pm1[:, 0 : SP - NCOL]
                )
                nc.vector.tensor_copy(out=t1[:, 0:NCOL], in_=P0[:, 0:NCOL])
                # t2 = t1 + shift_up(Pp1):  t2[x] = t1[x] + Pp1[x+64]
                nc.vector.tensor_add(
                    out=t2[:, 0 : SP - NCOL],
                    in0=t1[:, 0 : SP - NCOL],
                    in1=Pp1[:, NCOL:SP],
                )
                nc.vector.tensor_copy(
                    out=t2[:, SP - NCOL : SP], in_=t1[:, SP - NCOL : SP]
                )
                # y = tanh(t2 * mask + b * mask)  (mask in {0,1} zeroes inactive)
                nc.scalar.activation(
                    out=y[:],
                    in_=t2[:],
                    func=mybir.ActivationFunctionType.Tanh,
                    bias=biasm_sb[h0:h1, 0:1],
                    scale=mask_sb[h0:h1, 0:1],
                )
                nc.sync.dma_start(out=out[h0:h1, :], in_=y[:])

    if split_waits:
        _split_excess_waits(nc)
    return nc


_CACHE = {}


def _get_program():
    if "nc" not in _CACHE:
        _CACHE["nc"] = _build_program()
    return _CACHE["nc"]


def prep_in_maps(prev_outputs, prev_is_active, parent_indices, W, b):
    prev_outputs = np.asarray(prev_outputs, dtype=np.float32)
    prev_is_active = np.asarray(prev_is_active)
    parent_indices = np.asarray(parent_indices, dtype=np.int32)
    W = np.asarray(W, dtype=np.float32)
    b = np.asarray(b, dtype=np.float32)

    prev_flat = np.ascontiguousarray(prev_outputs.reshape(M_PREV, SP))

    # host: activity gate
    flags = prev_is_active.astype(np.int32)[parent_indices]  # (K, 16)
    out_active = flags.sum(axis=1) >= THRESHOLD  # (K,) bool

    in_maps = []
    for core in range(N_CORES):
        nodes = np.arange(core * NPC, (core + 1) * NPC)
        pi = parent_indices[nodes]  # (128, 16)
        # gather index layout: row r = n*16 + p for group's node n=r//16
        gidx = np.zeros((NPC, GROUPS), dtype=np.int32)
        for g in range(GROUPS):
            gnodes = pi[g * GN : (g + 1) * GN]  # (8, 16)
            gidx[:, g] = gnodes.reshape(NPC)
        # lhsT packing: (128, G*3*24); block-diagonal per group
        lhst = np.zeros((NPC, GROUPS * 3 * MCOL), dtype=np.float32)
        Wc = W[nodes]  # (128, 16, 3, 3)
        for g in range(GROUPS):
            for n in range(GN):
                node = g * GN + n
                for dxi in range(3):
                    for dyi in range(3):
                        # lhsT column (node-major): c = n*3 + dyi
                        c = (g * 3 + dxi) * MCOL + n * 3 + dyi
                        # lhst[row = n*16 + p, c] = W[node, p, dyi, dxi]
                        lhst[n * FAN_IN : (n + 1) * FAN_IN, c] = Wc[node, :, dyi, dxi]
        mask = out_active[nodes].astype(np.float32).reshape(NPC, 1)
        biasm = (b[nodes].reshape(NPC, 1) * mask).astype(np.float32)
        in_maps.append(
            {
                "prev": prev_flat,
                "gidx": gidx,
                "lhst": lhst,
                "biasm": biasm,
                "maskv": mask,
            }
        )
    return in_maps, out_active


def kernel(prev_outputs, prev_is_active, parent_indices, W, b):
    in_maps, out_active = prep_in_maps(
        prev_outputs, prev_is_active, parent_indices, W, b
    )
    nc = _get_program()
    try:
        res = run_bass_kernel_spmd(
            nc, in_maps, core_ids=list(range(N_CORES)), trace=bool(_CACHE.get("trace"))
        )
    except Exception:
        # transient NRT_EXEC_UNIT_UNRECOVERABLE has been observed once after
        # heavy compile churn; one retry clears it
        import time as _time

        _time.sleep(2.0)
        res = run_bass_kernel_spmd(
            nc, in_maps, core_ids=list(range(N_CORES)), trace=bool(_CACHE.get("trace"))
        )
    _CACHE["last_exec_ns"] = res.exec_time_ns
    _CACHE["last_profile"] = res.profile_json
    outs = [res.results[c]["out"].reshape(NPC, NROW, NCOL) for c in range(N_CORES)]
    out = np.concatenate(outs, axis=0)
    return out, out_active
